# revision 4
# baseline (speedup 1.0000x reference)
"""Radon adjoint (back-projection) as a Bass/Tile kernel on 8 NeuronCores.

Math per angle (exact grid_sample semantics, mirrors the validated jax
baseline): out(i,j) += wy * [P1 + e0*r0 + e2*m2] where
  P_k(i,j) = sp[off0 + B'_i + g'_j + k]     (one-hot matmuls, k=0,1,2)
  e0 = P0-P1, e2 = P2-P1
  w  = alpha_j + beta_i,  r0 = relu(1-w) = max(nalpha_j - beta_i, 0)
  m2 = relu(w-1) = -min(nalpha_j - beta_i, 0)
  wy = clip01(256.5 - |qr_i + qc_j|)

Angles in bucket X (|sin|<=|cos|) accumulate into accX in natural frame,
bucket Y into accY in transposed frame; out = (accX + accY^T)/720,
quantized to int8 with a host-provided scale.

Stage 1 (PE): Tz[m,j] = spw[m + g'_j] via Hankel-DMA'd H and one-hot ohg.
Stage 2 (PE): P_k[i,j] = sum_m [m == B'_i + k] Tz[m,j] via one-hot ohb.
Zero (angle,tile) matmuls are skipped using host-known index spans.
"""
import os
import numpy as np

import concourse.bass as bass
import concourse.mybir as mybir
from concourse.tile import TileContext
from concourse.masks import make_identity

B, A, D = 8, 720, 512
PADOFF, LSP = 600, 2048
HSLAB = 368          # H slab free length (m range 0..368)
F16 = mybir.dt.float16
F32 = mybir.dt.float32
I8 = mybir.dt.int8
AL = mybir.AluOpType
AF = mybir.ActivationFunctionType


def host_prep(angles_deg: np.ndarray):
    """Per-angle geometry in float64, matching the reference exactly."""
    ang = np.asarray(angles_deg, np.float64)
    rad = -np.deg2rad(ang)
    c, sn = np.cos(rad), np.sin(rad)
    jp = np.arange(D, dtype=np.float64) - 255.5
    geo = []
    for a in range(A):
        bx = abs(sn[a]) <= abs(c[a])
        if bx:
            u = c[a] * jp + (255.5 + PADOFF)
            v = -sn[a] * jp
            qr, qc = c[a] * jp, sn[a] * jp
        else:
            u = -sn[a] * jp + (255.5 + PADOFF)
            v = c[a] * jp
            qr, qc = sn[a] * jp, c[a] * jp
        g = np.floor(u)
        Bv = np.floor(v)
        alpha, beta = u - g, v - Bv
        gmin, bmin = g.min(), Bv.min()
        gp, bp = g - gmin, Bv - bmin
        off0 = int(bmin + gmin)
        gmax, bmax = int(gp.max()), int(bp.max())
        assert off0 >= 0 and off0 + 128 * ((gmax // 128) + 1) + HSLAB <= LSP
        geo.append(dict(
            bucket=0 if bx else 1, off0=off0,
            gp=gp, bp=bp, nalpha=1.0 - alpha, beta=beta, qr=qr, qc=qc,
            gmax=gmax, bmax=bmax,
        ))
    return geo


def build_program(nc, yq, ysc, osc, geo, angles):
    """Emit the Tile program. yq [1,720,512] int8, ysc [1,720] f32,
    osc [1,1] f32 (inv output scale, includes the 1/720). Returns out8."""
    out8 = nc.dram_tensor("out8", (1, D, D), I8, kind="ExternalOutput")

    # ---- inline geometry constants -------------------------------------
    gcb = nc.inline_tensor(np.stack([g["gp"] for g in geo]).astype(np.float16), "gcb")
    bcb = nc.inline_tensor(np.stack([g["bp"] for g in geo]).astype(np.float16), "bcb")
    nab = nc.inline_tensor(np.stack([g["nalpha"] for g in geo]).astype(np.float16), "nab")
    qcb = nc.inline_tensor(np.stack([g["qc"] for g in geo]).astype(np.float32), "qcb")
    colpack = np.zeros((A, 8, 128), np.float32)
    for a, g in enumerate(geo):
        colpack[a, 0:4] = g["beta"].reshape(4, 128)
        colpack[a, 4:8] = g["qr"].reshape(4, 128)
    colc = nc.inline_tensor(colpack, "colc")
    iotas = np.zeros((5, 128), np.float32)
    for t in range(5):
        iotas[t] = np.arange(128) + 128 * t
    iotc = nc.inline_tensor(iotas, "iotc")

    with TileContext(nc) as tc:
        import contextlib
        ctx = contextlib.ExitStack()
        with ctx:
            singles = ctx.enter_context(tc.tile_pool(name="singles", bufs=1))
            accp = ctx.enter_context(tc.tile_pool(name="acc", bufs=1))
            dram = ctx.enter_context(tc.tile_pool(name="dram", bufs=1, space="DRAM"))
            inp = ctx.enter_context(tc.tile_pool(name="inp", bufs=3))
            geop = ctx.enter_context(tc.tile_pool(name="geo", bufs=3))
            ohp = ctx.enter_context(tc.tile_pool(name="oh", bufs=3))
            ewp = ctx.enter_context(tc.tile_pool(name="ew", bufs=4))
            tzsp = ctx.enter_context(tc.tile_pool(name="tzs", bufs=6))
            tzps = ctx.enter_context(tc.tile_pool(name="tzps", bufs=2, space="PSUM"))
            pps = ctx.enter_context(tc.tile_pool(name="pps", bufs=2, space="PSUM"))
            endps = None  # allocated from tzps at the end

            # ---- setup -----------------------------------------------------
            iot = singles.tile([128, 5], F32)
            nc.sync.dma_start(out=iot, in_=bass.AP(
                tensor=iotc, offset=0, ap=[[1, 128], [128, 5]]))
            ident = singles.tile([128, 128], F32)
            make_identity(nc, ident)
            c2565 = singles.tile([128, 1], F32)
            nc.vector.memset(c2565, 256.5)
            oscc = singles.tile([128, 1], F32)
            nc.sync.dma_start(out=oscc, in_=bass.AP(
                tensor=osc, offset=0, ap=[[0, 128], [1, 1]]))

            accX = [accp.tile([128, D], F32, tag=f"accX{p}", name=f"accX{p}") for p in range(4)]
            accY = [accp.tile([128, D], F32, tag=f"accY{p}", name=f"accY{p}") for p in range(4)]
            for t in accX + accY:
                nc.vector.memset(t, 0.0)

            # sp_pad scratch in DRAM, zero + dequantized fp16 sinogram
            sp_pad = dram.tile([A, LSP], F16)
            zt = singles.tile([128, LSP], F16)
            nc.vector.memset(zt, 0.0)
            for rt in range(6):
                r0_, r1_ = 128 * rt, min(A, 128 * rt + 128)
                nr = r1_ - r0_
                nc.sync.dma_start(out=sp_pad[r0_:r1_, :], in_=zt[0:nr, :])
            for rt in range(6):
                r0_, r1_ = 128 * rt, min(A, 128 * rt + 128)
                nr = r1_ - r0_
                qt = inp.tile([128, D], I8, tag="qt")
                nc.sync.dma_start(out=qt[0:nr, :], in_=yq[0, r0_:r1_, :])
                sct = inp.tile([128, 1], F32, tag="sct")
                nc.sync.dma_start(out=sct[0:nr, :], in_=bass.AP(
                    tensor=ysc, offset=r0_, ap=[[1, nr], [1, 1]]))
                dqt = inp.tile([128, D], F16, tag="dqt")
                nc.scalar.mul(dqt[0:nr, :], qt[0:nr, :], sct[0:nr, 0:1])
                nc.sync.dma_start(out=sp_pad[r0_:r1_, PADOFF:PADOFF + D],
                                  in_=dqt[0:nr, :])

            # ---- per-angle pipeline ---------------------------------------
            for a in angles:
                g = geo[a]
                acc = accX if g["bucket"] == 0 else accY
                ngt = g["gmax"] // 128 + 1          # q-tiles with any hits
                mneed = g["bmax"] + 3               # Tz rows required
                nmt = (mneed + 127) // 128          # m-tiles
                bp_ = g["bp"]

                # H: [q', t, m] = sp_pad[a, off0 + 128t + q' + m]
                H = geop.tile([128, 5 * HSLAB], F16, tag="H")
                nc.sync.dma_start(
                    out=H[:, 0:ngt * HSLAB],
                    in_=bass.AP(tensor=sp_pad.tensor,
                                offset=sp_pad.offset + a * LSP + g["off0"],
                                ap=[[1, 128], [128, ngt], [1, HSLAB]]))

                def bcast(const, dt, tag):
                    tt = geop.tile([128, D], dt, tag=tag, name=tag)
                    nc.sync.dma_start(out=tt, in_=bass.AP(
                        tensor=const, offset=a * D, ap=[[0, 128], [1, D]]))
                    return tt
                g_bc = bcast(gcb, F16, "g_bc")
                b_bc = bcast(bcb, F16, "b_bc")
                na_bc = bcast(nab, F16, "na_bc")
                qc_bc = bcast(qcb, F32, "qc_bc")
                cols = geop.tile([128, 8], F32, tag="cols")
                nc.sync.dma_start(out=cols, in_=bass.AP(
                    tensor=colc, offset=a * 8 * 128, ap=[[1, 128], [128, 8]]))

                # stage 1: Tz m-tiles -> SBUF fp16
                ohg = []
                for t in range(ngt):
                    o = ohp.tile([128, D], F16, tag=f"ohg{t}", name=f"ohg{t}")
                    nc.vector.tensor_scalar(
                        out=o, in0=g_bc, scalar1=iot[:, t:t + 1], scalar2=None,
                        op0=AL.is_equal)
                    ohg.append(o)
                tzs = []
                for mt in range(nmt):
                    mlen = min(128, mneed - 128 * mt)
                    tzp = tzps.tile([128, D], F32, tag="tzp")
                    for t in range(ngt):
                        nc.tensor.matmul(
                            tzp[0:mlen, :],
                            H[:, t * HSLAB + 128 * mt:t * HSLAB + 128 * mt + mlen],
                            ohg[t], start=(t == 0), stop=(t == ngt - 1))
                    ts_ = tzsp.tile([128, D], F16, tag="tzs")
                    nc.scalar.copy(ts_[0:mlen, :], tzp[0:mlen, :])
                    tzs.append((ts_, mlen))

                # ohb builds for (k, mt) actually used
                used = {}
                for p in range(4):
                    bsl = bp_[128 * p:128 * p + 128]
                    bmn, bmx = int(bsl.min()), int(bsl.max())
                    for k in range(3):
                        used[(p, k)] = list(
                            range((bmn + k) // 128, (bmx + k) // 128 + 1))
                ohb = {}
                for (p, k), mts in used.items():
                    for mt in mts:
                        if (k, mt) not in ohb:
                            o = ohp.tile([128, D], F16, tag=f"ohb{k}{mt}", name=f"ohb{k}{mt}")
                            nc.gpsimd.tensor_scalar(
                                out=o, in0=b_bc, scalar1=iot[:, mt:mt + 1],
                                scalar2=-float(k), op0=AL.subtract,
                                op1=AL.is_equal)
                            ohb[(k, mt)] = o
                # differenced one-hots: D01 = ohb0-ohb1 (-> E0 = P0-P1),
                # D21 = ohb2-ohb1 (-> E2 = P2-P1); zero-fill missing taps
                mts_by_p = {p: sorted(set(used[(p, 0)] + used[(p, 1)]
                                          + used[(p, 2)])) for p in range(4)}
                all_mts = sorted(set(m for v in mts_by_p.values() for m in v))
                zoh = None
                D01, D21 = {}, {}
                for mt in all_mts:
                    def get(k):
                        nonlocal zoh
                        if (k, mt) in ohb:
                            return ohb[(k, mt)]
                        if zoh is None:
                            zoh = ohp.tile([128, D], F16, tag="zoh", name="zoh")
                            nc.gpsimd.memset(zoh, 0.0)
                        return zoh
                    d0 = ohp.tile([128, D], F16, tag=f"d01{mt}", name=f"d01{mt}")
                    nc.gpsimd.tensor_tensor(out=d0, in0=get(0), in1=get(1),
                                            op=AL.subtract)
                    D01[mt] = d0
                    d2 = ohp.tile([128, D], F16, tag=f"d21{mt}", name=f"d21{mt}")
                    nc.gpsimd.tensor_tensor(out=d2, in0=get(2), in1=get(1),
                                            op=AL.subtract)
                    D21[mt] = d2

                # stage 2 + elementwise per i-tile
                for p in range(4):
                    mts = mts_by_p[p]
                    E0 = pps.tile([128, D], F32, tag="E0", name="E0")
                    E2 = pps.tile([128, D], F32, tag="E2", name="E2")
                    P1 = pps.tile([128, D], F32, tag="P1", name="P1")
                    for i_, mt in enumerate(mts):
                        mlen = tzs[mt][1]
                        st, sp_ = (i_ == 0), (i_ == len(mts) - 1)
                        psl = slice(128 * p, 128 * p + 128)
                        nc.tensor.matmul(E0, D01[mt][0:mlen, psl],
                                         tzs[mt][0][0:mlen, :], start=st, stop=sp_)
                        nc.tensor.matmul(E2, D21[mt][0:mlen, psl],
                                         tzs[mt][0][0:mlen, :], start=st, stop=sp_)
                        if mt in used[(p, 1)]:
                            pst = (mt == used[(p, 1)][0])
                            psp = (mt == used[(p, 1)][-1])
                            nc.tensor.matmul(P1, ohb[(1, mt)][0:mlen, psl],
                                             tzs[mt][0][0:mlen, :],
                                             start=pst, stop=psp)
                    bcol = cols[:, p:p + 1]
                    qrcol = cols[:, 4 + p:5 + p]
                    r0 = ewp.tile([128, D], F16, tag="r0")
                    nc.gpsimd.tensor_scalar(out=r0, in0=na_bc, scalar1=bcol,
                                            scalar2=0.0, op0=AL.subtract,
                                            op1=AL.max)
                    vv = ewp.tile([128, D], F16, tag="vv")
                    nc.gpsimd.tensor_scalar(out=vv, in0=na_bc, scalar1=bcol,
                                            scalar2=0.0, op0=AL.subtract,
                                            op1=AL.min)
                    t1 = ewp.tile([128, D], F16, tag="t1")
                    nc.vector.tensor_tensor(out=t1, in0=r0, in1=E0, op=AL.mult)
                    t2 = ewp.tile([128, D], F16, tag="t2")
                    nc.vector.scalar_tensor_tensor(out=t2, in0=vv, scalar=-1.0,
                                                   in1=E2, op0=AL.mult,
                                                   op1=AL.mult)
                    s12 = ewp.tile([128, D], F16, tag="s12")
                    nc.gpsimd.tensor_tensor(out=s12, in0=t1, in1=t2, op=AL.add)
                    sP = ewp.tile([128, D], F16, tag="sP")
                    nc.vector.tensor_tensor(out=sP, in0=s12, in1=P1,
                                            op=AL.add)
                    aq = ewp.tile([128, D], F32, tag="aq")
                    nc.scalar.activation(out=aq, in_=qc_bc, func=AF.Abs,
                                         bias=qrcol, scale=1.0)
                    w0 = ewp.tile([128, D], F16, tag="w0")
                    nc.scalar.activation(out=w0, in_=aq, func=AF.Relu,
                                         bias=c2565[:, 0:1], scale=-1.0)
                    wys = ewp.tile([128, D], F32, tag="wys")
                    nc.vector.scalar_tensor_tensor(out=wys, in0=w0, scalar=1.0,
                                                   in1=sP, op0=AL.min,
                                                   op1=AL.mult)
                    nc.gpsimd.dma_start(out=acc[p], in_=wys,
                                        accum_op=AL.add)

            # ---- finalize: out = (accX + accY^T) * osc -> int8 -------------
            for p in range(4):
                q8 = ewp.tile([128, D], I8, tag="q8")
                for t in range(4):
                    tp = tzps.tile([128, 128], F32, tag="tzp")
                    nc.tensor.transpose(tp, accY[t][:, 128 * p:128 * p + 128],
                                        ident)
                    u = ewp.tile([128, 128], F32, tag="uadd")
                    nc.vector.tensor_tensor(out=u,
                                            in0=accX[p][:, 128 * t:128 * t + 128],
                                            in1=tp, op=AL.add)
                    nc.vector.tensor_scalar(out=q8[:, 128 * t:128 * t + 128],
                                            in0=u, scalar1=oscc[:, 0:1],
                                            scalar2=None, op0=AL.mult)
                nc.sync.dma_start(out=out8[0, 128 * p:128 * p + 128, :], in_=q8)
    return out8


# ======================= runner / public entry =======================
import time
import jax
from jax.sharding import Mesh, PartitionSpec as _P

for _k, _v in (("jax_compilation_cache_dir", "/tmp/jax_cache"),
               ("jax_persistent_cache_min_entry_size_bytes", -1),
               ("jax_persistent_cache_min_compile_time_secs", 0.0)):
    try:
        jax.config.update(_k, _v)
    except Exception:
        pass

from concourse.bass2jax import bass_jit, bass_shard_map

C_BOUND = 0.33   # calibrated max|out| <= C * rms(y_b); observed 0.243 worst
_MAGIC = np.float32(12582912.0)  # 1.5 * 2**23: fast round-to-nearest for f32

_mesh = Mesh(np.array(jax.devices()[:B]), ("d",))
_FN_CACHE = {}


def _get_fn(angles_deg: np.ndarray):
    key = angles_deg.tobytes()
    fn = _FN_CACHE.get(key)
    if fn is not None:
        return fn
    geo = host_prep(angles_deg)
    ang_list = list(range(A))

    @bass_jit
    def _bp(nc, yq, ysc, osc):
        return build_program(nc, yq, ysc, osc, geo, ang_list)

    fn = bass_shard_map(_bp, mesh=_mesh,
                        in_specs=(_P("d"), _P("d"), _P("d")),
                        out_specs=_P("d"))
    _FN_CACHE[key] = fn
    return fn


def kernel(y: np.ndarray, angles_deg: np.ndarray) -> np.ndarray:
    t0 = time.perf_counter()
    y = np.asarray(y, np.float32)
    angles_deg = np.asarray(angles_deg, np.float32)
    fn = _get_fn(angles_deg)
    t1 = time.perf_counter()

    yr = y.reshape(B, A, D)
    rowmax = np.abs(yr).max(axis=2)                       # [B,A]
    ysc = np.maximum(rowmax, 1e-30) * np.float32(1.0 / 127.0)
    yq = ((yr * (1.0 / ysc)[:, :, None] + _MAGIC) - _MAGIC).astype(np.int8)
    # rms from a subsample (output-scale bound has a 1.36x margin)
    ys = yr[:, ::7, :]
    rms = np.sqrt((ys * ys).mean(axis=(1, 2)))
    scale_out = (C_BOUND * rms / 127.0).astype(np.float32)
    osc = (1.0 / (A * scale_out)).reshape(B, 1).astype(np.float32)
    t2 = time.perf_counter()

    q8 = np.asarray(fn(yq, ysc.astype(np.float32), osc))  # [B,512,512] int8
    t3 = time.perf_counter()
    res = q8.astype(np.float32)
    res *= scale_out[:, None, None]
    t4 = time.perf_counter()
    if os.environ.get("BP_DEBUG"):
        print(f"[bp] prep {t1-t0:.3f}s quant {t2-t1:.3f}s "
              f"exec+fetch {t3-t2:.3f}s deq {t4-t3:.3f}s")
    return res[:, None].astype(np.float32)


if __name__ == "__main__":
    rng = np.random.default_rng(0)
    _y = rng.standard_normal((B, 1, A, D)).astype(np.float32)
    _ang = np.linspace(0.0, 180.0, A + 1, dtype=np.float32)[:-1]
    _out = kernel(_y, _ang)
    print(_out.shape, _out.dtype, float(np.abs(_out).mean()))


# revision 5
# speedup vs baseline: 1.1919x; 1.1919x over previous
"""Radon adjoint (back-projection) as a Bass/Tile kernel on 8 NeuronCores.

Math per angle (exact grid_sample semantics, mirrors the validated jax
baseline): out(i,j) += wy * [P1 + e0*r0 + e2*m2] where
  P_k(i,j) = sp[off0 + B'_i + g'_j + k]     (one-hot matmuls, k=0,1,2)
  e0 = P0-P1, e2 = P2-P1
  w  = alpha_j + beta_i,  r0 = relu(1-w) = max(nalpha_j - beta_i, 0)
  m2 = relu(w-1) = -min(nalpha_j - beta_i, 0)
  wy = clip01(256.5 - |qr_i + qc_j|)

Angles in bucket X (|sin|<=|cos|) accumulate into accX in natural frame,
bucket Y into accY in transposed frame; out = (accX + accY^T)/720,
quantized to int8 with a host-provided scale.

Stage 1 (PE): Tz[m,j] = spw[m + g'_j] via Hankel-DMA'd H and one-hot ohg.
Stage 2 (PE): P_k[i,j] = sum_m [m == B'_i + k] Tz[m,j] via one-hot ohb.
Zero (angle,tile) matmuls are skipped using host-known index spans.
"""
import os
import numpy as np

import concourse.bass as bass
import concourse.mybir as mybir
from concourse.tile import TileContext
from concourse.masks import make_identity

B, A, D = 8, 720, 512
PADOFF, LSP = 600, 2048
HSLAB = 368          # H slab free length (m range 0..368)
F16 = mybir.dt.float16
F32 = mybir.dt.float32
I8 = mybir.dt.int8
AL = mybir.AluOpType
AF = mybir.ActivationFunctionType


def host_prep(angles_deg: np.ndarray):
    """Per-angle geometry in float64, matching the reference exactly."""
    ang = np.asarray(angles_deg, np.float64)
    rad = -np.deg2rad(ang)
    c, sn = np.cos(rad), np.sin(rad)
    jp = np.arange(D, dtype=np.float64) - 255.5
    geo = []
    for a in range(A):
        bx = abs(sn[a]) <= abs(c[a])
        if bx:
            u = c[a] * jp + (255.5 + PADOFF)
            v = -sn[a] * jp
            qr, qc = c[a] * jp, sn[a] * jp
        else:
            u = -sn[a] * jp + (255.5 + PADOFF)
            v = c[a] * jp
            qr, qc = sn[a] * jp, c[a] * jp
        g = np.floor(u)
        Bv = np.floor(v)
        alpha, beta = u - g, v - Bv
        gmin, bmin = g.min(), Bv.min()
        gp, bp = g - gmin, Bv - bmin
        off0 = int(bmin + gmin)
        gmax, bmax = int(gp.max()), int(bp.max())
        assert off0 >= 0 and off0 + 128 * ((gmax // 128) + 1) + HSLAB <= LSP
        geo.append(dict(
            bucket=0 if bx else 1, off0=off0,
            gp=gp, bp=bp, nalpha=1.0 - alpha, beta=beta, qr=qr, qc=qc,
            gmax=gmax, bmax=bmax,
        ))
    return geo


def build_program(nc, yq, ysc, osc, geo, angles):
    """Emit the Tile program. yq [1,720,512] int8, ysc [1,720] f32,
    osc [1,1] f32 (inv output scale, includes the 1/720). Returns out8."""
    out8 = nc.dram_tensor("out8", (1, D, D), I8, kind="ExternalOutput")

    # ---- inline geometry constants -------------------------------------
    gcb = nc.inline_tensor(np.stack([g["gp"] for g in geo]).astype(np.float16), "gcb")
    bcb = nc.inline_tensor(np.stack([g["bp"] for g in geo]).astype(np.float16), "bcb")
    nab = nc.inline_tensor(np.stack([g["nalpha"] for g in geo]).astype(np.float16), "nab")
    qcb = nc.inline_tensor(np.stack([g["qc"] for g in geo]).astype(np.float32), "qcb")
    colpack = np.zeros((A, 8, 128), np.float32)
    for a, g in enumerate(geo):
        colpack[a, 0:4] = g["beta"].reshape(4, 128)
        colpack[a, 4:8] = g["qr"].reshape(4, 128)
    colc = nc.inline_tensor(colpack, "colc")
    iotas = np.zeros((5, 128), np.float32)
    for t in range(5):
        iotas[t] = np.arange(128) + 128 * t
    iotc = nc.inline_tensor(iotas, "iotc")

    with TileContext(nc) as tc:
        import contextlib
        ctx = contextlib.ExitStack()
        with ctx:
            singles = ctx.enter_context(tc.tile_pool(name="singles", bufs=1))
            accp = ctx.enter_context(tc.tile_pool(name="acc", bufs=1))
            dram = ctx.enter_context(tc.tile_pool(name="dram", bufs=1, space="DRAM"))
            inp = ctx.enter_context(tc.tile_pool(name="inp", bufs=3))
            geop = ctx.enter_context(tc.tile_pool(name="geo", bufs=3))
            ohp = ctx.enter_context(tc.tile_pool(name="oh", bufs=3))
            ewp = ctx.enter_context(tc.tile_pool(name="ew", bufs=4))
            tzsp = ctx.enter_context(tc.tile_pool(name="tzs", bufs=6))
            tzps = ctx.enter_context(tc.tile_pool(name="tzps", bufs=2, space="PSUM"))
            pps = ctx.enter_context(tc.tile_pool(name="pps", bufs=2, space="PSUM"))
            endps = None  # allocated from tzps at the end

            # ---- setup -----------------------------------------------------
            iot = singles.tile([128, 5], F32)
            nc.sync.dma_start(out=iot, in_=bass.AP(
                tensor=iotc, offset=0, ap=[[1, 128], [128, 5]]))
            ident = singles.tile([128, 128], F32)
            make_identity(nc, ident)
            c2565 = singles.tile([128, 1], F32)
            nc.vector.memset(c2565, 256.5)
            oscc = singles.tile([128, 1], F32)
            nc.sync.dma_start(out=oscc, in_=bass.AP(
                tensor=osc, offset=0, ap=[[0, 128], [1, 1]]))

            accX = [accp.tile([128, D], F32, tag=f"accX{p}", name=f"accX{p}") for p in range(4)]
            accY = [accp.tile([128, D], F32, tag=f"accY{p}", name=f"accY{p}") for p in range(4)]
            for t in accX + accY:
                nc.vector.memset(t, 0.0)

            # sp_pad scratch in DRAM, zero + dequantized fp16 sinogram
            sp_pad = dram.tile([A, LSP], F16)
            zt = singles.tile([128, LSP], F16)
            nc.vector.memset(zt, 0.0)
            for rt in range(6):
                r0_, r1_ = 128 * rt, min(A, 128 * rt + 128)
                nr = r1_ - r0_
                nc.sync.dma_start(out=sp_pad[r0_:r1_, :], in_=zt[0:nr, :])
            for rt in range(6):
                r0_, r1_ = 128 * rt, min(A, 128 * rt + 128)
                nr = r1_ - r0_
                qt = inp.tile([128, D], I8, tag="qt")
                nc.sync.dma_start(out=qt[0:nr, :], in_=yq[0, r0_:r1_, :])
                sct = inp.tile([128, 1], F32, tag="sct")
                nc.sync.dma_start(out=sct[0:nr, :], in_=bass.AP(
                    tensor=ysc, offset=r0_, ap=[[1, nr], [1, 1]]))
                dqt = inp.tile([128, D], F16, tag="dqt")
                nc.scalar.mul(dqt[0:nr, :], qt[0:nr, :], sct[0:nr, 0:1])
                nc.sync.dma_start(out=sp_pad[r0_:r1_, PADOFF:PADOFF + D],
                                  in_=dqt[0:nr, :])

            # ---- per-angle pipeline ---------------------------------------
            for a in angles:
                g = geo[a]
                acc = accX if g["bucket"] == 0 else accY
                ngt = g["gmax"] // 128 + 1          # q-tiles with any hits
                mneed = g["bmax"] + 3               # Tz rows required
                nmt = (mneed + 127) // 128          # m-tiles
                bp_ = g["bp"]

                # H: [q', t, m] = sp_pad[a, off0 + 128t + q' + m]
                H = geop.tile([128, 5 * HSLAB], F16, tag="H")
                nc.sync.dma_start(
                    out=H[:, 0:ngt * HSLAB],
                    in_=bass.AP(tensor=sp_pad.tensor,
                                offset=sp_pad.offset + a * LSP + g["off0"],
                                ap=[[1, 128], [128, ngt], [1, HSLAB]]))

                def bcast(const, dt, tag):
                    tt = geop.tile([128, D], dt, tag=tag, name=tag)
                    nc.sync.dma_start(out=tt, in_=bass.AP(
                        tensor=const, offset=a * D, ap=[[0, 128], [1, D]]))
                    return tt
                g_bc = bcast(gcb, F16, "g_bc")
                b_bc = bcast(bcb, F16, "b_bc")
                na_bc = bcast(nab, F16, "na_bc")
                qc_bc = bcast(qcb, F32, "qc_bc")
                cols = geop.tile([128, 8], F32, tag="cols")
                nc.sync.dma_start(out=cols, in_=bass.AP(
                    tensor=colc, offset=a * 8 * 128, ap=[[1, 128], [128, 8]]))

                # stage 1: Tz m-tiles -> SBUF fp16
                ohg = []
                for t in range(ngt):
                    o = ohp.tile([128, D], F16, tag=f"ohg{t}", name=f"ohg{t}")
                    nc.vector.tensor_scalar(
                        out=o, in0=g_bc, scalar1=iot[:, t:t + 1], scalar2=None,
                        op0=AL.is_equal)
                    ohg.append(o)
                tzs = []
                for mt in range(nmt):
                    mlen = min(128, mneed - 128 * mt)
                    tzp = tzps.tile([128, D], F32, tag="tzp")
                    for t in range(ngt):
                        nc.tensor.matmul(
                            tzp[0:mlen, :],
                            H[:, t * HSLAB + 128 * mt:t * HSLAB + 128 * mt + mlen],
                            ohg[t], start=(t == 0), stop=(t == ngt - 1))
                    ts_ = tzsp.tile([128, D], F16, tag="tzs")
                    nc.scalar.copy(ts_[0:mlen, :], tzp[0:mlen, :])
                    tzs.append((ts_, mlen))

                # ohb builds for (k, mt) actually used
                used = {}
                for p in range(4):
                    bsl = bp_[128 * p:128 * p + 128]
                    bmn, bmx = int(bsl.min()), int(bsl.max())
                    for k in range(3):
                        used[(p, k)] = list(
                            range((bmn + k) // 128, (bmx + k) // 128 + 1))
                ohb = {}
                for (p, k), mts in used.items():
                    for mt in mts:
                        if (k, mt) not in ohb:
                            o = ohp.tile([128, D], F16, tag=f"ohb{k}{mt}", name=f"ohb{k}{mt}")
                            nc.gpsimd.tensor_scalar(
                                out=o, in0=b_bc, scalar1=iot[:, mt:mt + 1],
                                scalar2=-float(k), op0=AL.subtract,
                                op1=AL.is_equal)
                            ohb[(k, mt)] = o
                # differenced one-hots: D01 = ohb0-ohb1 (-> E0 = P0-P1),
                # D21 = ohb2-ohb1 (-> E2 = P2-P1); zero-fill missing taps
                mts_by_p = {p: sorted(set(used[(p, 0)] + used[(p, 1)]
                                          + used[(p, 2)])) for p in range(4)}
                all_mts = sorted(set(m for v in mts_by_p.values() for m in v))
                zoh = None
                D01, D21 = {}, {}
                for mt in all_mts:
                    def get(k):
                        nonlocal zoh
                        if (k, mt) in ohb:
                            return ohb[(k, mt)]
                        if zoh is None:
                            zoh = ohp.tile([128, D], F16, tag="zoh", name="zoh")
                            nc.gpsimd.memset(zoh, 0.0)
                        return zoh
                    d0 = ohp.tile([128, D], F16, tag=f"d01{mt}", name=f"d01{mt}")
                    nc.gpsimd.tensor_tensor(out=d0, in0=get(0), in1=get(1),
                                            op=AL.subtract)
                    D01[mt] = d0
                    d2 = ohp.tile([128, D], F16, tag=f"d21{mt}", name=f"d21{mt}")
                    nc.gpsimd.tensor_tensor(out=d2, in0=get(2), in1=get(1),
                                            op=AL.subtract)
                    D21[mt] = d2

                # stage 2 + elementwise per i-tile
                for p in range(4):
                    mts = mts_by_p[p]
                    E0 = pps.tile([128, D], F32, tag="E0", name="E0")
                    E2 = pps.tile([128, D], F32, tag="E2", name="E2")
                    P1 = pps.tile([128, D], F32, tag="P1", name="P1")
                    for i_, mt in enumerate(mts):
                        mlen = tzs[mt][1]
                        st, sp_ = (i_ == 0), (i_ == len(mts) - 1)
                        psl = slice(128 * p, 128 * p + 128)
                        nc.tensor.matmul(E0, D01[mt][0:mlen, psl],
                                         tzs[mt][0][0:mlen, :], start=st, stop=sp_)
                        nc.tensor.matmul(E2, D21[mt][0:mlen, psl],
                                         tzs[mt][0][0:mlen, :], start=st, stop=sp_)
                        if mt in used[(p, 1)]:
                            pst = (mt == used[(p, 1)][0])
                            psp = (mt == used[(p, 1)][-1])
                            nc.tensor.matmul(P1, ohb[(1, mt)][0:mlen, psl],
                                             tzs[mt][0][0:mlen, :],
                                             start=pst, stop=psp)
                    bcol = cols[:, p:p + 1]
                    qrcol = cols[:, 4 + p:5 + p]
                    r0 = ewp.tile([128, D], F16, tag="r0")
                    nc.gpsimd.tensor_scalar(out=r0, in0=na_bc, scalar1=bcol,
                                            scalar2=0.0, op0=AL.subtract,
                                            op1=AL.max)
                    vv = ewp.tile([128, D], F16, tag="vv")
                    nc.gpsimd.tensor_scalar(out=vv, in0=na_bc, scalar1=bcol,
                                            scalar2=0.0, op0=AL.subtract,
                                            op1=AL.min)
                    t1 = ewp.tile([128, D], F16, tag="t1")
                    nc.vector.tensor_tensor(out=t1, in0=r0, in1=E0, op=AL.mult)
                    t2 = ewp.tile([128, D], F16, tag="t2")
                    nc.vector.scalar_tensor_tensor(out=t2, in0=vv, scalar=-1.0,
                                                   in1=E2, op0=AL.mult,
                                                   op1=AL.mult)
                    s12 = ewp.tile([128, D], F16, tag="s12")
                    nc.gpsimd.tensor_tensor(out=s12, in0=t1, in1=t2, op=AL.add)
                    sP = ewp.tile([128, D], F16, tag="sP")
                    nc.vector.tensor_tensor(out=sP, in0=s12, in1=P1,
                                            op=AL.add)
                    aq = ewp.tile([128, D], F32, tag="aq")
                    nc.scalar.activation(out=aq, in_=qc_bc, func=AF.Abs,
                                         bias=qrcol, scale=1.0)
                    w0 = ewp.tile([128, D], F16, tag="w0")
                    nc.scalar.activation(out=w0, in_=aq, func=AF.Relu,
                                         bias=c2565[:, 0:1], scale=-1.0)
                    wys = ewp.tile([128, D], F32, tag="wys")
                    nc.vector.scalar_tensor_tensor(out=wys, in0=w0, scalar=1.0,
                                                   in1=sP, op0=AL.min,
                                                   op1=AL.mult)
                    nc.vector.tensor_tensor(out=acc[p], in0=acc[p], in1=wys,
                                            op=AL.add)

            # ---- finalize: out = (accX + accY^T) * osc -> int8 -------------
            for p in range(4):
                q8 = ewp.tile([128, D], I8, tag="q8")
                for t in range(4):
                    tp = tzps.tile([128, 128], F32, tag="tzp")
                    nc.tensor.transpose(tp, accY[t][:, 128 * p:128 * p + 128],
                                        ident)
                    u = ewp.tile([128, 128], F32, tag="uadd")
                    nc.vector.tensor_tensor(out=u,
                                            in0=accX[p][:, 128 * t:128 * t + 128],
                                            in1=tp, op=AL.add)
                    nc.vector.tensor_scalar(out=q8[:, 128 * t:128 * t + 128],
                                            in0=u, scalar1=oscc[:, 0:1],
                                            scalar2=None, op0=AL.mult)
                nc.sync.dma_start(out=out8[0, 128 * p:128 * p + 128, :], in_=q8)
    return out8


# ======================= runner / public entry =======================
import time
import jax
from jax.sharding import Mesh, PartitionSpec as _P

for _k, _v in (("jax_compilation_cache_dir", "/tmp/jax_cache"),
               ("jax_persistent_cache_min_entry_size_bytes", -1),
               ("jax_persistent_cache_min_compile_time_secs", 0.0)):
    try:
        jax.config.update(_k, _v)
    except Exception:
        pass

from concourse.bass2jax import bass_jit, bass_shard_map

C_BOUND = 0.33   # calibrated max|out| <= C * rms(y_b); observed 0.243 worst
_MAGIC = np.float32(12582912.0)  # 1.5 * 2**23: fast round-to-nearest for f32

_mesh = Mesh(np.array(jax.devices()[:B]), ("d",))
_FN_CACHE = {}


def _get_fn(angles_deg: np.ndarray):
    key = angles_deg.tobytes()
    fn = _FN_CACHE.get(key)
    if fn is not None:
        return fn
    geo = host_prep(angles_deg)
    ang_list = list(range(A))

    @bass_jit
    def _bp(nc, yq, ysc, osc):
        return build_program(nc, yq, ysc, osc, geo, ang_list)

    fn = bass_shard_map(_bp, mesh=_mesh,
                        in_specs=(_P("d"), _P("d"), _P("d")),
                        out_specs=_P("d"))
    _FN_CACHE[key] = fn
    return fn


def kernel(y: np.ndarray, angles_deg: np.ndarray) -> np.ndarray:
    t0 = time.perf_counter()
    y = np.asarray(y, np.float32)
    angles_deg = np.asarray(angles_deg, np.float32)
    fn = _get_fn(angles_deg)
    t1 = time.perf_counter()

    yr = y.reshape(B, A, D)
    rowmax = np.abs(yr).max(axis=2)                       # [B,A]
    ysc = np.maximum(rowmax, 1e-30) * np.float32(1.0 / 127.0)
    yq = ((yr * (1.0 / ysc)[:, :, None] + _MAGIC) - _MAGIC).astype(np.int8)
    # rms from a subsample (output-scale bound has a 1.36x margin)
    ys = yr[:, ::7, :]
    rms = np.sqrt((ys * ys).mean(axis=(1, 2)))
    scale_out = (C_BOUND * rms / 127.0).astype(np.float32)
    osc = (1.0 / (A * scale_out)).reshape(B, 1).astype(np.float32)
    t2 = time.perf_counter()

    q8 = np.asarray(fn(yq, ysc.astype(np.float32), osc))  # [B,512,512] int8
    t3 = time.perf_counter()
    res = q8.astype(np.float32)
    res *= scale_out[:, None, None]
    t4 = time.perf_counter()
    if os.environ.get("BP_DEBUG"):
        print(f"[bp] prep {t1-t0:.3f}s quant {t2-t1:.3f}s "
              f"exec+fetch {t3-t2:.3f}s deq {t4-t3:.3f}s")
    return res[:, None].astype(np.float32)


if __name__ == "__main__":
    rng = np.random.default_rng(0)
    _y = rng.standard_normal((B, 1, A, D)).astype(np.float32)
    _ang = np.linspace(0.0, 180.0, A + 1, dtype=np.float32)[:-1]
    _out = kernel(_y, _ang)
    print(_out.shape, _out.dtype, float(np.abs(_out).mean()))


# revision 6
# speedup vs baseline: 1.7195x; 1.4427x over previous
"""Radon adjoint (back-projection) as a Bass/Tile kernel on 8 NeuronCores.

Math per angle (exact grid_sample semantics, mirrors the validated jax
baseline): out(i,j) += wy * [P1 + e0*r0 + e2*m2] where
  P_k(i,j) = sp[off0 + B'_i + g'_j + k]     (one-hot matmuls, k=0,1,2)
  e0 = P0-P1, e2 = P2-P1
  w  = alpha_j + beta_i,  r0 = relu(1-w) = max(nalpha_j - beta_i, 0)
  m2 = relu(w-1) = -min(nalpha_j - beta_i, 0)
  wy = clip01(256.5 - |qr_i + qc_j|)

Angles in bucket X (|sin|<=|cos|) accumulate into accX in natural frame,
bucket Y into accY in transposed frame; out = (accX + accY^T)/720,
quantized to int8 with a host-provided scale.

Stage 1 (PE): Tz[m,j] = spw[m + g'_j] via Hankel-DMA'd H and one-hot ohg.
Stage 2 (PE): P_k[i,j] = sum_m [m == B'_i + k] Tz[m,j] via one-hot ohb.
Zero (angle,tile) matmuls are skipped using host-known index spans.
"""
import os
import numpy as np

import concourse.bass as bass
import concourse.mybir as mybir
from concourse.tile import TileContext
from concourse.masks import make_identity

B, A, D = 8, 720, 512
PADOFF, LSP = 600, 2048
HSLAB = 368          # H slab free length (m range 0..368)
F16 = mybir.dt.float16
F32 = mybir.dt.float32
I8 = mybir.dt.int8
AL = mybir.AluOpType
AF = mybir.ActivationFunctionType


def host_prep(angles_deg: np.ndarray):
    """Per-angle geometry in float64, matching the reference exactly."""
    ang = np.asarray(angles_deg, np.float64)
    rad = -np.deg2rad(ang)
    c, sn = np.cos(rad), np.sin(rad)
    jp = np.arange(D, dtype=np.float64) - 255.5
    geo = []
    for a in range(A):
        bx = abs(sn[a]) <= abs(c[a])
        if bx:
            u = c[a] * jp + (255.5 + PADOFF)
            v = -sn[a] * jp
            qr, qc = c[a] * jp, sn[a] * jp
        else:
            u = -sn[a] * jp + (255.5 + PADOFF)
            v = c[a] * jp
            qr, qc = sn[a] * jp, c[a] * jp
        g = np.floor(u)
        Bv = np.floor(v)
        alpha, beta = u - g, v - Bv
        gmin, bmin = g.min(), Bv.min()
        gp, bp = g - gmin, Bv - bmin
        off0 = int(bmin + gmin)
        gmax, bmax = int(gp.max()), int(bp.max())
        assert off0 >= 0 and off0 + 128 * ((gmax // 128) + 1) + HSLAB <= LSP
        geo.append(dict(
            bucket=0 if bx else 1, off0=off0,
            gp=gp, bp=bp, nalpha=1.0 - alpha, beta=beta, qr=qr, qc=qc,
            gmax=gmax, bmax=bmax,
        ))
    return geo


def build_program(nc, yq, ysc, osc, geo, angles):
    """Emit the Tile program. yq [1,720,512] int8, ysc [1,720] f32,
    osc [1,1] f32 (inv output scale, includes the 1/720). Returns out8."""
    out8 = nc.dram_tensor("out8", (1, D, D), I8, kind="ExternalOutput")

    # ---- inline geometry constants -------------------------------------
    gcb = nc.inline_tensor(np.stack([g["gp"] for g in geo]).astype(np.float16), "gcb")
    bcb = nc.inline_tensor(np.stack([g["bp"] for g in geo]).astype(np.float16), "bcb")
    nab = nc.inline_tensor(np.stack([g["nalpha"] for g in geo]).astype(np.float16), "nab")
    qcb = nc.inline_tensor(np.stack([g["qc"] for g in geo]).astype(np.float32), "qcb")
    colpack = np.zeros((A, 8, 128), np.float32)
    for a, g in enumerate(geo):
        colpack[a, 0:4] = g["beta"].reshape(4, 128)
        colpack[a, 4:8] = g["qr"].reshape(4, 128)
    colc = nc.inline_tensor(colpack, "colc")
    iotas = np.zeros((5, 128), np.float32)
    for t in range(5):
        iotas[t] = np.arange(128) + 128 * t
    iotc = nc.inline_tensor(iotas, "iotc")

    with TileContext(nc) as tc:
        import contextlib
        ctx = contextlib.ExitStack()
        with ctx:
            singles = ctx.enter_context(tc.tile_pool(name="singles", bufs=1))
            accp = ctx.enter_context(tc.tile_pool(name="acc", bufs=1))
            dram = ctx.enter_context(tc.tile_pool(name="dram", bufs=1, space="DRAM"))
            inp = ctx.enter_context(tc.tile_pool(name="inp", bufs=3))
            geop = ctx.enter_context(tc.tile_pool(name="geo", bufs=3))
            ohp = ctx.enter_context(tc.tile_pool(name="oh", bufs=3))
            ewp = ctx.enter_context(tc.tile_pool(name="ew", bufs=4))
            tzsp = ctx.enter_context(tc.tile_pool(name="tzs", bufs=6))
            tzps = ctx.enter_context(tc.tile_pool(name="tzps", bufs=2, space="PSUM"))
            pps = ctx.enter_context(tc.tile_pool(name="pps", bufs=2, space="PSUM"))
            endps = None  # allocated from tzps at the end

            # ---- setup -----------------------------------------------------
            iot = singles.tile([128, 5], F32)
            nc.sync.dma_start(out=iot, in_=bass.AP(
                tensor=iotc, offset=0, ap=[[1, 128], [128, 5]]))
            ident = singles.tile([128, 128], F32)
            make_identity(nc, ident)
            c2565 = singles.tile([128, 1], F32)
            nc.vector.memset(c2565, 256.5)
            oscc = singles.tile([128, 1], F32)
            nc.sync.dma_start(out=oscc, in_=bass.AP(
                tensor=osc, offset=0, ap=[[0, 128], [1, 1]]))

            accX = [accp.tile([128, D], F32, tag=f"accX{p}", name=f"accX{p}") for p in range(4)]
            accY = [accp.tile([128, D], F32, tag=f"accY{p}", name=f"accY{p}") for p in range(4)]
            for t in accX + accY:
                nc.vector.memset(t, 0.0)

            # sp_pad scratch in DRAM, zero + dequantized fp16 sinogram
            sp_pad = dram.tile([A, LSP], F16)
            zt = singles.tile([128, LSP], F16)
            nc.vector.memset(zt, 0.0)
            for rt in range(6):
                r0_, r1_ = 128 * rt, min(A, 128 * rt + 128)
                nr = r1_ - r0_
                nc.sync.dma_start(out=sp_pad[r0_:r1_, :], in_=zt[0:nr, :])
            for rt in range(6):
                r0_, r1_ = 128 * rt, min(A, 128 * rt + 128)
                nr = r1_ - r0_
                qt = inp.tile([128, D], I8, tag="qt")
                nc.sync.dma_start(out=qt[0:nr, :], in_=yq[0, r0_:r1_, :])
                sct = inp.tile([128, 1], F32, tag="sct")
                nc.sync.dma_start(out=sct[0:nr, :], in_=bass.AP(
                    tensor=ysc, offset=r0_, ap=[[1, nr], [1, 1]]))
                dqt = inp.tile([128, D], F16, tag="dqt")
                nc.scalar.mul(dqt[0:nr, :], qt[0:nr, :], sct[0:nr, 0:1])
                nc.sync.dma_start(out=sp_pad[r0_:r1_, PADOFF:PADOFF + D],
                                  in_=dqt[0:nr, :])

            # ---- per-angle pipeline ---------------------------------------
            for a in angles:
                g = geo[a]
                acc = accX if g["bucket"] == 0 else accY
                ngt = g["gmax"] // 128 + 1          # q-tiles with any hits
                mneed = g["bmax"] + 3               # Tz rows required
                nmt = (mneed + 127) // 128          # m-tiles
                bp_ = g["bp"]

                # H: [q', t, m] = sp_pad[a, off0 + 128t + q' + m]
                H = geop.tile([128, 5 * HSLAB], F16, tag="H")
                nc.sync.dma_start(
                    out=H[:, 0:ngt * HSLAB],
                    in_=bass.AP(tensor=sp_pad.tensor,
                                offset=sp_pad.offset + a * LSP + g["off0"],
                                ap=[[1, 128], [128, ngt], [1, HSLAB]]))

                def bcast(const, dt, tag):
                    tt = geop.tile([128, D], dt, tag=tag, name=tag)
                    nc.sync.dma_start(out=tt, in_=bass.AP(
                        tensor=const, offset=a * D, ap=[[0, 128], [1, D]]))
                    return tt
                g_bc = bcast(gcb, F16, "g_bc")
                b_bc = bcast(bcb, F16, "b_bc")
                na_bc = bcast(nab, F16, "na_bc")
                qc_bc = bcast(qcb, F32, "qc_bc")
                cols = geop.tile([128, 8], F32, tag="cols")
                nc.sync.dma_start(out=cols, in_=bass.AP(
                    tensor=colc, offset=a * 8 * 128, ap=[[1, 128], [128, 8]]))

                # stage 1: Tz m-tiles -> SBUF fp16
                ohg = []
                for t in range(ngt):
                    o = ohp.tile([128, D], F16, tag=f"ohg{t}", name=f"ohg{t}")
                    nc.vector.tensor_scalar(
                        out=o, in0=g_bc, scalar1=iot[:, t:t + 1], scalar2=None,
                        op0=AL.is_equal)
                    ohg.append(o)
                tzs = []
                for mt in range(nmt):
                    mlen = min(128, mneed - 128 * mt)
                    tzp = tzps.tile([128, D], F32, tag="tzp")
                    for t in range(ngt):
                        nc.tensor.matmul(
                            tzp[0:mlen, :],
                            H[:, t * HSLAB + 128 * mt:t * HSLAB + 128 * mt + mlen],
                            ohg[t], start=(t == 0), stop=(t == ngt - 1))
                    ts_ = tzsp.tile([128, D], F16, tag="tzs")
                    nc.scalar.copy(ts_[0:mlen, :], tzp[0:mlen, :])
                    tzs.append((ts_, mlen))

                # ohb builds for (k, mt) actually used
                used = {}
                for p in range(4):
                    bsl = bp_[128 * p:128 * p + 128]
                    bmn, bmx = int(bsl.min()), int(bsl.max())
                    for k in range(3):
                        used[(p, k)] = list(
                            range((bmn + k) // 128, (bmx + k) // 128 + 1))
                ohb = {}
                for (p, k), mts in used.items():
                    for mt in mts:
                        if (k, mt) not in ohb:
                            o = ohp.tile([128, D], F16, tag=f"ohb{k}{mt}", name=f"ohb{k}{mt}")
                            nc.vector.tensor_scalar(
                                out=o, in0=b_bc, scalar1=iot[:, mt:mt + 1],
                                scalar2=-float(k), op0=AL.subtract,
                                op1=AL.is_equal)
                            ohb[(k, mt)] = o
                # differenced one-hots: D01 = ohb0-ohb1 (-> E0 = P0-P1),
                # D21 = ohb2-ohb1 (-> E2 = P2-P1); zero-fill missing taps
                mts_by_p = {p: sorted(set(used[(p, 0)] + used[(p, 1)]
                                          + used[(p, 2)])) for p in range(4)}
                all_mts = sorted(set(m for v in mts_by_p.values() for m in v))
                zoh = None
                D01, D21 = {}, {}
                for mt in all_mts:
                    def get(k):
                        nonlocal zoh
                        if (k, mt) in ohb:
                            return ohb[(k, mt)]
                        if zoh is None:
                            zoh = ohp.tile([128, D], F16, tag="zoh", name="zoh")
                            nc.vector.memset(zoh, 0.0)
                        return zoh
                    d0 = ohp.tile([128, D], F16, tag=f"d01{mt}", name=f"d01{mt}")
                    nc.vector.tensor_tensor(out=d0, in0=get(0), in1=get(1),
                                            op=AL.subtract)
                    D01[mt] = d0
                    d2 = ohp.tile([128, D], F16, tag=f"d21{mt}", name=f"d21{mt}")
                    nc.vector.tensor_tensor(out=d2, in0=get(2), in1=get(1),
                                            op=AL.subtract)
                    D21[mt] = d2

                # stage 2 + elementwise per i-tile
                for p in range(4):
                    mts = mts_by_p[p]
                    E0 = pps.tile([128, D], F32, tag="E0", name="E0")
                    E2 = pps.tile([128, D], F32, tag="E2", name="E2")
                    P1 = pps.tile([128, D], F32, tag="P1", name="P1")
                    for i_, mt in enumerate(mts):
                        mlen = tzs[mt][1]
                        st, sp_ = (i_ == 0), (i_ == len(mts) - 1)
                        psl = slice(128 * p, 128 * p + 128)
                        nc.tensor.matmul(E0, D01[mt][0:mlen, psl],
                                         tzs[mt][0][0:mlen, :], start=st, stop=sp_)
                        nc.tensor.matmul(E2, D21[mt][0:mlen, psl],
                                         tzs[mt][0][0:mlen, :], start=st, stop=sp_)
                        if mt in used[(p, 1)]:
                            pst = (mt == used[(p, 1)][0])
                            psp = (mt == used[(p, 1)][-1])
                            nc.tensor.matmul(P1, ohb[(1, mt)][0:mlen, psl],
                                             tzs[mt][0][0:mlen, :],
                                             start=pst, stop=psp)
                    bcol = cols[:, p:p + 1]
                    qrcol = cols[:, 4 + p:5 + p]
                    r0 = ewp.tile([128, D], F16, tag="r0")
                    nc.vector.tensor_scalar(out=r0, in0=na_bc, scalar1=bcol,
                                            scalar2=0.0, op0=AL.subtract,
                                            op1=AL.max)
                    vv = ewp.tile([128, D], F16, tag="vv")
                    nc.vector.tensor_scalar(out=vv, in0=na_bc, scalar1=bcol,
                                            scalar2=0.0, op0=AL.subtract,
                                            op1=AL.min)
                    t1 = ewp.tile([128, D], F16, tag="t1")
                    nc.vector.tensor_tensor(out=t1, in0=r0, in1=E0, op=AL.mult)
                    t2 = ewp.tile([128, D], F16, tag="t2")
                    nc.vector.scalar_tensor_tensor(out=t2, in0=vv, scalar=-1.0,
                                                   in1=E2, op0=AL.mult,
                                                   op1=AL.mult)
                    s12 = ewp.tile([128, D], F16, tag="s12")
                    nc.vector.tensor_tensor(out=s12, in0=t1, in1=t2, op=AL.add)
                    sP = ewp.tile([128, D], F16, tag="sP")
                    nc.vector.tensor_tensor(out=sP, in0=s12, in1=P1,
                                            op=AL.add)
                    aq = ewp.tile([128, D], F32, tag="aq")
                    nc.scalar.activation(out=aq, in_=qc_bc, func=AF.Abs,
                                         bias=qrcol, scale=1.0)
                    w0 = ewp.tile([128, D], F16, tag="w0")
                    nc.scalar.activation(out=w0, in_=aq, func=AF.Relu,
                                         bias=c2565[:, 0:1], scale=-1.0)
                    wys = ewp.tile([128, D], F32, tag="wys")
                    nc.vector.scalar_tensor_tensor(out=wys, in0=w0, scalar=1.0,
                                                   in1=sP, op0=AL.min,
                                                   op1=AL.mult)
                    nc.vector.tensor_tensor(out=acc[p], in0=acc[p], in1=wys,
                                            op=AL.add)

            # ---- finalize: out = (accX + accY^T) * osc -> int8 -------------
            for p in range(4):
                q8 = ewp.tile([128, D], I8, tag="q8")
                for t in range(4):
                    tp = tzps.tile([128, 128], F32, tag="tzp")
                    nc.tensor.transpose(tp, accY[t][:, 128 * p:128 * p + 128],
                                        ident)
                    u = ewp.tile([128, 128], F32, tag="uadd")
                    nc.vector.tensor_tensor(out=u,
                                            in0=accX[p][:, 128 * t:128 * t + 128],
                                            in1=tp, op=AL.add)
                    nc.vector.tensor_scalar(out=q8[:, 128 * t:128 * t + 128],
                                            in0=u, scalar1=oscc[:, 0:1],
                                            scalar2=None, op0=AL.mult)
                nc.sync.dma_start(out=out8[0, 128 * p:128 * p + 128, :], in_=q8)
    return out8


# ======================= runner / public entry =======================
import time
import jax
from jax.sharding import Mesh, PartitionSpec as _P

for _k, _v in (("jax_compilation_cache_dir", "/tmp/jax_cache"),
               ("jax_persistent_cache_min_entry_size_bytes", -1),
               ("jax_persistent_cache_min_compile_time_secs", 0.0)):
    try:
        jax.config.update(_k, _v)
    except Exception:
        pass

from concourse.bass2jax import bass_jit, bass_shard_map

C_BOUND = 0.33   # calibrated max|out| <= C * rms(y_b); observed 0.243 worst
_MAGIC = np.float32(12582912.0)  # 1.5 * 2**23: fast round-to-nearest for f32

_mesh = Mesh(np.array(jax.devices()[:B]), ("d",))
_FN_CACHE = {}


def _get_fn(angles_deg: np.ndarray):
    key = angles_deg.tobytes()
    fn = _FN_CACHE.get(key)
    if fn is not None:
        return fn
    geo = host_prep(angles_deg)
    ang_list = list(range(A))

    @bass_jit
    def _bp(nc, yq, ysc, osc):
        return build_program(nc, yq, ysc, osc, geo, ang_list)

    fn = bass_shard_map(_bp, mesh=_mesh,
                        in_specs=(_P("d"), _P("d"), _P("d")),
                        out_specs=_P("d"))
    _FN_CACHE[key] = fn
    return fn


def kernel(y: np.ndarray, angles_deg: np.ndarray) -> np.ndarray:
    t0 = time.perf_counter()
    y = np.asarray(y, np.float32)
    angles_deg = np.asarray(angles_deg, np.float32)
    fn = _get_fn(angles_deg)
    t1 = time.perf_counter()

    yr = y.reshape(B, A, D)
    rowmax = np.abs(yr).max(axis=2)                       # [B,A]
    ysc = np.maximum(rowmax, 1e-30) * np.float32(1.0 / 127.0)
    yq = ((yr * (1.0 / ysc)[:, :, None] + _MAGIC) - _MAGIC).astype(np.int8)
    # rms from a subsample (output-scale bound has a 1.36x margin)
    ys = yr[:, ::7, :]
    rms = np.sqrt((ys * ys).mean(axis=(1, 2)))
    scale_out = (C_BOUND * rms / 127.0).astype(np.float32)
    osc = (1.0 / (A * scale_out)).reshape(B, 1).astype(np.float32)
    t2 = time.perf_counter()

    q8 = np.asarray(fn(yq, ysc.astype(np.float32), osc))  # [B,512,512] int8
    t3 = time.perf_counter()
    res = q8.astype(np.float32)
    res *= scale_out[:, None, None]
    t4 = time.perf_counter()
    if os.environ.get("BP_DEBUG"):
        print(f"[bp] prep {t1-t0:.3f}s quant {t2-t1:.3f}s "
              f"exec+fetch {t3-t2:.3f}s deq {t4-t3:.3f}s")
    return res[:, None].astype(np.float32)


if __name__ == "__main__":
    rng = np.random.default_rng(0)
    _y = rng.standard_normal((B, 1, A, D)).astype(np.float32)
    _ang = np.linspace(0.0, 180.0, A + 1, dtype=np.float32)[:-1]
    _out = kernel(_y, _ang)
    print(_out.shape, _out.dtype, float(np.abs(_out).mean()))
